# revision 24
# baseline (speedup 1.0000x reference)
"""Trainium2 Bass kernel for nn_GNNet_5420248728073 (GNN message passing).

Strategy (8 NeuronCores, SPMD):
  - Nodes are sharded into 8 blocks of 192 (N=1500 padded to 1536); core k owns
    the edges whose *target* lies in its block, laid out on a per-node slot grid
    [192 nodes x L slots] so segment-max becomes a fixed-stride tensor_reduce.
  - Per loop iteration each core computes fx messages for its edges, reduces to
    agg over its own nodes (all edges of a target live on one core), updates its
    x block, then AllGathers the x blocks (as bf16 main+residual rows) into a
    replicated node table, from which x[src] is fetched with a transposed
    dma_gather (feature-major).  x[tgt] is a broadcast expand of the local
    block (exact f32).  fy then updates the per-edge state y.
  - Precision: gathered x travels as bf16 (m) + bf16 residual (r); matmuls use
    split weights so the result is f32-accurate (~1e-5 rel).  Everything else
    (h, second layers, y, msg, agg, x) is f32.
  - Output: y rows (feature-major) + x block per core; the host unshard places
    the 24000 y rows into the (zero) dense [N, N, 64] edge_feat.
"""

import numpy as np
import ml_dtypes

E = 64            # feature dim
NB = 192          # nodes per core block
CORES = 8
NPAD = NB * CORES # 1536


def _bf16(a):
    return np.asarray(a, np.float32).astype(ml_dtypes.bfloat16)


def _wrap_idx(idx):
    """dma_gather index layout: [128, n/16] int16, idx i at [i%16, i//16],
    replicated across the 8 gpsimd cores (16 partitions each)."""
    n = idx.shape[0]
    assert n % 16 == 0
    base = idx.reshape(n // 16, 16).T.astype(np.int16)   # [16, n/16]
    return np.tile(base, (8, 1))                          # [128, n/16]


def _build_bass_program(L, LOOP, stage=99):
    LM, LX = 24, 16
    import concourse.bass as bass
    import concourse.bacc as bacc
    import concourse.tile as tile
    import concourse.mybir as mybir
    from concourse.masks import make_identity

    NI = NB * LM + 32 * LX
    NCH = NI // 512
    assert NI % 512 == 0 and NI % 128 == 0

    f32 = mybir.dt.float32
    bf = mybir.dt.bfloat16
    AF = mybir.ActivationFunctionType
    OP = mybir.AluOpType

    nc = bacc.Bacc("TRN2", target_bir_lowering=False, debug=False,
                   num_devices=CORES)

    # ---- I/O ----
    vct = nc.dram_tensor("vct", [NPAD, 128], bf, kind="ExternalInput")
    sgidx = nc.dram_tensor("sgidx", [128, NI // 16], mybir.dt.int16, kind="ExternalInput")
    vcpT = nc.dram_tensor("vcpT", [8, NB], f32, kind="ExternalInput")
    goal = nc.dram_tensor("goal", [8, 1], f32, kind="ExternalInput")
    wb = nc.dram_tensor("wb", [128, 384], bf, kind="ExternalInput")
    wf = nc.dram_tensor("wf", [64, 832], f32, kind="ExternalInput")
    wf2 = nc.dram_tensor("wf2", [128, 64], f32, kind="ExternalInput")
    bias = nc.dram_tensor("bias", [128, 16], f32, kind="ExternalInput")
    y_out = nc.dram_tensor("y_out", [64, NI], f32, kind="ExternalOutput")
    x_out = nc.dram_tensor("x_out", [64, NB], f32, kind="ExternalOutput")

    with tile.TileContext(nc) as tc:
        with (
            tc.tile_pool(name="persist", bufs=1) as pp,
            tc.tile_pool(name="rot", bufs=2) as rot,
            tc.tile_pool(name="work", bufs=3) as wk,
            tc.tile_pool(name="psum", bufs=2, space="PSUM") as ps,
            tc.tile_pool(name="dram", bufs=2, space="DRAM") as dr,
        ):
            # ---- load constants ----
            wb_sb = pp.tile([128, 384], bf, tag="wb")
            nc.sync.dma_start(wb_sb[:], wb[:])
            wf_sb = pp.tile([64, 832], f32, tag="wf")
            nc.sync.dma_start(wf_sb[:], wf[:])
            wf_sb2 = pp.tile([128, 64], f32, tag="wf2")
            nc.sync.dma_start(wf_sb2[:], wf2[:])
            b_sb = pp.tile([128, 16], f32, tag="bias")
            nc.sync.dma_start(b_sb[:], bias[:])
            idx_sb = pp.tile([128, NI // 16], mybir.dt.int16, tag="idx")
            nc.sync.dma_start(idx_sb[:], sgidx[:])
            vcT_sb = pp.tile([8, NB], f32, tag="vcT")
            nc.sync.dma_start(vcT_sb[:], vcpT[:])
            goal_sb = pp.tile([8, 1], f32, tag="goal")
            nc.sync.dma_start(goal_sb[:], goal[:])
            ident = pp.tile([128, 128], bf, tag="ident")
            make_identity(nc, ident[:])

            W_fx1 = wb_sb[:, 0:64]
            W_fx2 = wb_sb[:, 64:128]
            W_fy1 = wb_sb[:, 128:192]
            W_fy2 = wb_sb[:, 192:256]
            W_hy1 = wb_sb[0:16, 256:320]
            W_hy2 = wb_sb[0:8, 320:384]
            W_BD = wf_sb2[:, 0:64]
            W_Ap = wf_sb[:, 128:192]
            W_2f = wf_sb[:, 192:256]
            W_2g = wf_sb[:, 256:320]
            W_2x = wf_sb[:, 320:384]
            W_2y = wf_sb[:, 384:448]
            W_Ah = wf_sb[0:8, 512:576]
            W_1xa = wf_sb[0:8, 576:640]
            W_1xb = wf_sb[0:8, 640:704]
            W_1xc = wf_sb[0:8, 704:768]
            W_1xd = wf_sb[0:8, 768:832]
            b1f = b_sb[0:64, 0:1]
            b1g = b_sb[0:64, 1:2]
            b2f = b_sb[0:64, 2:3]
            b1x = b_sb[0:64, 3:4]
            b2x = b_sb[0:64, 4:5]
            b1y = b_sb[0:64, 5:6]
            b2ys = b_sb[64:128, 6:7]
            b2g_hi = b_sb[64:128, 7:8]

            # ---- persistent state ----
            x_blk = pp.tile([64, NB], f32, tag="x")        # f32 x of my block
            U = pp.tile([128, NI], f32, tag="U")           # [T=x[tgt]; y-b2g]
            grid = pp.tile([128, NI], f32, tag="grid")     # msg slots (lo half)
            agg = pp.tile([64, NB], f32, tag="agg")
            agg_e = pp.tile([64, 32], f32, tag="agge")
            NMAIN = NB * LM                                # main slots
            assert NMAIN + 32 * LX == NI

            # ---- x0 = mlp_hx([vc, goal, d, d*d]) for my block ----
            t_goal = pp.tile([8, NB], f32, tag="tgoal")
            nc.vector.tensor_copy(t_goal[:], goal_sb[:].to_broadcast([8, NB]))
            t_d = pp.tile([8, NB], f32, tag="td")
            nc.vector.tensor_scalar_sub(t_d[:], vcT_sb[:], goal_sb[:])
            t_dd = pp.tile([8, NB], f32, tag="tdd")
            nc.vector.tensor_tensor(out=t_dd[:], in0=t_d[:], in1=t_d[:], op=OP.mult)
            ph = ps.tile([64, NB], f32, tag="pa")
            nc.tensor.matmul(ph[:], W_1xa, vcT_sb[:], start=True, stop=False)
            nc.tensor.matmul(ph[:], W_1xb, t_goal[:], start=False, stop=False)
            nc.tensor.matmul(ph[:], W_1xc, t_d[:], start=False, stop=False)
            nc.tensor.matmul(ph[:], W_1xd, t_dd[:], start=False, stop=True)
            h0 = wk.tile([64, NB], f32, tag="h")
            nc.scalar.activation(h0[:], ph[:], AF.Relu, bias=b1x)
            px = ps.tile([64, NB], f32, tag="pb")
            nc.tensor.matmul(px[:], W_2x, h0[:], start=True, stop=True)
            nc.scalar.activation(x_blk[:], px[:], AF.Identity, bias=b2x)

            def x_chain():
                """cast x_blk -> [m|r] node-major rows, AllGather into the
                replicated table, gather S1=[m;r][src] and expand T=x[tgt]."""
                xm = wk.tile([64, NB], bf, tag="xm")
                nc.scalar.activation(xm[:], x_blk[:], AF.Copy)
                xr_f = wk.tile([64, NB], f32, tag="xrf")
                nc.vector.tensor_tensor(out=xr_f[:], in0=x_blk[:], in1=xm[:],
                                        op=OP.subtract)
                xr = wk.tile([64, NB], bf, tag="xr")
                nc.scalar.activation(xr[:], xr_f[:], AF.Copy)
                nmA = wk.tile([128, 128], bf, tag="nmA")
                nmB = wk.tile([64, 128], bf, tag="nmB")
                for srcap, dlo in ((xm, 0), (xr, 64)):
                    pt = ps.tile([128, 64], bf, tag="pa")
                    nc.tensor.transpose(pt[0:128, 0:64], srcap[:, 0:128],
                                        ident[0:64, 0:64])
                    nc.vector.tensor_copy(nmA[:, dlo:dlo + 64], pt[0:128, 0:64])
                    pt2 = ps.tile([64, 64], bf, tag="pb")
                    nc.tensor.transpose(pt2[0:64, 0:64], srcap[:, 128:192],
                                        ident[0:64, 0:64])
                    nc.vector.tensor_copy(nmB[:, dlo:dlo + 64], pt2[0:64, 0:64])
                agin = dr.tile([NB, 128], bf, tag="agin")
                nc.sync.dma_start(agin[0:128, :], nmA[:])
                nc.sync.dma_start(agin[128:192, :], nmB[:])
                table = dr.tile([NPAD, 128], bf, tag="table")
                nc.gpsimd.collective_compute(
                    "AllGather", OP.bypass,
                    replica_groups=[list(range(CORES))],
                    ins=[agin.opt()], outs=[table.opt()],
                )
                S1 = rot.tile([128, NI], bf, tag="S1")
                for lo, hi in ((0, 1536), (1536, NI)):
                    nc.gpsimd.dma_gather(
                        S1[:, lo:hi].rearrange("p (a n) -> p a n", a=1),
                        table[:], idx_sb[:, lo // 16:hi // 16],
                        hi - lo, hi - lo, 128, transpose=True, single_packet=False,
                    )
                nc.vector.tensor_copy(
                    U[0:64, 0:NMAIN].rearrange("p (n l) -> p n l", l=LM),
                    x_blk[:, :, None].to_broadcast([64, NB, LM]),
                )
                nc.vector.tensor_copy(
                    U[0:64, NMAIN:NI].rearrange("p (n l) -> p n l", l=LX),
                    x_blk[:, 0:32, None].to_broadcast([64, 32, LX]),
                )
                return S1

            # ---- y0 (uses vc table: [vm|vr] in first 16 halfwords) ----
            G0 = rot.tile([128, NI], bf, tag="S1")
            H = NI // 2
            for hh in range(2):
                nc.gpsimd.dma_gather(
                    G0[:, hh * H:(hh + 1) * H].rearrange("p (a n) -> p a n", a=1),
                    vct[:], idx_sb[:, hh * (H // 16):(hh + 1) * (H // 16)],
                    H, H, 128, transpose=True, single_packet=False,
                )
            T0 = pp.tile([8, NI], f32, tag="T0")
            nc.vector.tensor_copy(
                T0[:, 0:NMAIN].rearrange("p (n l) -> p n l", l=LM),
                vcT_sb[:, :, None].to_broadcast([8, NB, LM]),
            )
            nc.vector.tensor_copy(
                T0[:, NMAIN:NI].rearrange("p (n l) -> p n l", l=LX),
                vcT_sb[:, 0:32, None].to_broadcast([8, 32, LX]),
            )
            for c in range(NCH):
                cs = slice(c * 512, (c + 1) * 512)
                p1 = ps.tile([64, 512], f32, tag="pa")
                nc.tensor.matmul(p1[:], W_hy1, G0[0:16, cs], start=True, stop=False)
                nc.tensor.matmul(p1[:], W_hy2, G0[0:8, cs], start=False, stop=False)
                nc.tensor.matmul(p1[:], W_Ah, T0[:, cs], start=False, stop=True)
                hh = wk.tile([64, 512], f32, tag="h")
                nc.scalar.activation(hh[:], p1[:], AF.Relu, bias=b1y)
                p2 = ps.tile([128, 512], f32, tag="pb")
                nc.tensor.matmul(p2[64:128, :], W_2y, hh[:], start=True, stop=True,
                                 tile_position=(0, 64))
                nc.scalar.activation(U[64:128, cs], p2[64:128, :], AF.Identity,
                                     bias=b2ys)

            S1 = x_chain()

            # ---- main loop ----
            for t in range(LOOP if stage >= 6 else 0):
                # fx: msg into grid (lo half)
                for c in range(NCH):
                    cs = slice(c * 512, (c + 1) * 512)
                    p1 = ps.tile([64, 512], f32, tag="pa")
                    nc.tensor.matmul(p1[:], W_fx1, S1[:, cs], start=True, stop=False)
                    nc.tensor.matmul(p1[:], W_fx2, S1[:, cs], start=False, stop=False)
                    nc.tensor.matmul(p1[:], W_BD, U[:, cs], start=False, stop=True)
                    hh = wk.tile([64, 512], f32, tag="h")
                    nc.scalar.activation(hh[:], p1[:], AF.Relu, bias=b1f)
                    p2 = ps.tile([64, 512], f32, tag="pb")
                    nc.tensor.matmul(p2[:], W_2f, hh[:], start=True, stop=True)
                    nc.scalar.activation(grid[0:64, cs], p2[:], AF.Identity, bias=b2f)
                # segment max + x update (main grid + overflow tier)
                nc.vector.tensor_reduce(
                    agg[:], grid[0:64, 0:NMAIN].rearrange("p (n l) -> p n l", l=LM),
                    axis=mybir.AxisListType.X, op=OP.max,
                )
                nc.vector.tensor_reduce(
                    agg_e[:], grid[0:64, NMAIN:NI].rearrange("p (n l) -> p n l", l=LX),
                    axis=mybir.AxisListType.X, op=OP.max,
                )
                nc.vector.tensor_tensor(out=agg[:, 0:32], in0=agg[:, 0:32],
                                        in1=agg_e[:], op=OP.max)
                nc.vector.tensor_tensor(out=x_blk[:], in0=x_blk[:], in1=agg[:],
                                        op=OP.max)
                # distribute x_{t+1}; gathers serve fy_t and fx_{t+1}
                S1 = x_chain()
                # fy: y = max(y, mlp)
                for c in range(NCH):
                    cs = slice(c * 512, (c + 1) * 512)
                    p3 = ps.tile([64, 512], f32, tag="pc")
                    nc.tensor.matmul(p3[:], W_Ap, U[0:64, cs], start=True, stop=False)
                    nc.tensor.matmul(p3[:], W_fy1, S1[:, cs], start=False, stop=False)
                    nc.tensor.matmul(p3[:], W_fy2, S1[:, cs], start=False, stop=True)
                    hh = wk.tile([64, 512], f32, tag="h")
                    nc.scalar.activation(hh[:], p3[:], AF.Relu, bias=b1g)
                    p4 = ps.tile([128, 512], f32, tag="pd")
                    nc.tensor.matmul(p4[64:128, :], W_2g, hh[:], start=True, stop=True,
                                     tile_position=(0, 64))
                    nc.vector.tensor_tensor(out=U[64:128, cs], in0=U[64:128, cs],
                                            in1=p4[64:128, :], op=OP.max)

            # ---- outputs ----
            nc.scalar.activation(grid[64:128, :], U[64:128, :], AF.Identity,
                                 bias=b2g_hi)
            nc.sync.dma_start(y_out[:], grid[64:128, :])
            nc.sync.dma_start(x_out[:], x_blk[:])

    nc.compile()
    return nc


_CACHE = {}
PROFILE = False       # set True (before calling kernel) to capture an NTFF trace
LAST_RESULT = None    # BassKernelResults of the last run


def _host_prep(inputs):
    inp = {k: np.asarray(v) for k, v in inputs.items()}
    v = inp["v"].astype(np.float32)
    labels = inp["labels"].astype(np.float32)
    edge_index = inp["edge_index"].astype(np.int64)
    LOOP = int(inp["loop"])
    N = v.shape[0]
    src, tgt = edge_index[0], edge_index[1]

    vc = np.concatenate([v, labels], -1)                    # [N, 8]
    goal = vc[int(np.argmax(labels[:, 0]))]
    vcp = np.zeros((NPAD, 8), np.float32)
    vcp[:N] = vc

    # per-target-node edge lists; two-tier slot grid:
    #   main: LM slots per node; ext: LX extra slots for local nodes 0..31
    #   (nodes are relabeled per block by degree descending)
    LM, LX = 24, 16
    NI = NB * LM + 32 * LX
    NMAIN = NB * LM
    deg = np.bincount(tgt, minlength=NPAD)
    assert int(deg.max()) <= LM + LX

    order = np.argsort(tgt, kind="stable")
    starts = np.zeros(NPAD + 1, np.int64)
    np.cumsum(deg, out=starts[1:])

    slot_src = np.zeros((CORES, NI), np.int64)
    slot_real = np.full((CORES, NI), -1, np.int64)   # original edge id or -1
    perm = np.zeros((CORES, NB), np.int64)           # local id -> global node
    local_of = np.zeros(NPAD, np.int64)              # global -> table row
    for k in range(CORES):
        blk_deg = deg[k * NB:(k + 1) * NB]
        assert (blk_deg > LM).sum() <= 32
        rank = np.argsort(-blk_deg, kind="stable")   # local id ln = rank order
        perm[k] = k * NB + rank
        local_of[perm[k]] = k * NB + np.arange(NB)
        for ln in range(NB):
            g = int(perm[k, ln])
            es = order[starts[g]:starts[g + 1]]
            dn = len(es)
            s0 = ln * LM
            if dn == 0:
                slot_src[k, s0:s0 + LM] = g
                continue
            n_main = min(dn, LM)
            slot_src[k, s0:s0 + n_main] = src[es[:n_main]]
            slot_real[k, s0:s0 + n_main] = es[:n_main]
            slot_src[k, s0 + n_main:s0 + LM] = src[es[0]]
            if ln < 32:
                e0 = NMAIN + ln * LX
                n_ext = dn - n_main
                slot_src[k, e0:e0 + LX] = src[es[0]]
                if n_ext > 0:
                    slot_src[k, e0:e0 + n_ext] = src[es[n_main:]]
                    slot_real[k, e0:e0 + n_ext] = es[n_main:]
            else:
                assert dn <= LM
    # gather indices address the (relabeled) table rows
    slot_src = local_of[slot_src]

    # vc main+residual gather table, rows in table (relabeled) order
    flat_perm = perm.reshape(-1)
    vcp_l = vcp[flat_perm]
    vm = _bf16(vcp_l)
    vr = _bf16(vcp_l - vm.astype(np.float32))
    vct_np = np.zeros((NPAD, 128), ml_dtypes.bfloat16)
    vct_np[:, 0:8] = vm
    vct_np[:, 8:16] = vr

    # weights
    W1x, b1x, W2x, b2x = inp["hx_W1"], inp["hx_b1"], inp["hx_W2"], inp["hx_b2"]
    W1y, b1y, W2y, b2y = inp["hy_W1"], inp["hy_b1"], inp["hy_W2"], inp["hy_b2"]
    W1f, b1f, W2f, b2f = inp["fx_W1"], inp["fx_b1"], inp["fx_W2"], inp["fx_b2"]
    W1g, b1g, W2g, b2g = inp["fy_W1"], inp["fy_b1"], inp["fy_W2"], inp["fy_b2"]
    A = W1f[64:128] + W1f[0:64]
    B = W1f[128:192] - W1f[0:64]
    D = W1f[192:256]
    Ap = W1g[0:64] + W1g[64:128]
    Bp = W1g[128:192] - W1g[0:64]
    Ah = W1y[0:8] + W1y[8:16]
    Bh = W1y[16:24] - W1y[0:8]

    Am = _bf16(A); Ar = _bf16(A - Am.astype(np.float32))
    Bpm = _bf16(Bp); Bpr = _bf16(Bp - Bpm.astype(np.float32))
    Bhm = _bf16(Bh); Bhr = _bf16(Bh - Bhm.astype(np.float32))

    wb_np = np.zeros((128, 384), ml_dtypes.bfloat16)
    wb_np[:, 0:64] = np.concatenate([Am, Am], 0)
    wb_np[:, 64:128] = np.concatenate([Ar, Ar], 0)
    wb_np[:, 128:192] = np.concatenate([Bpm, Bpm], 0)
    wb_np[:, 192:256] = np.concatenate([Bpr, Bpr], 0)
    wb_np[0:16, 256:320] = np.concatenate([Bhm, Bhm], 0)
    wb_np[0:8, 320:384] = Bhr

    wf_np = np.zeros((64, 832), np.float32)
    wf2_np = np.ascontiguousarray(np.concatenate([B, D], 0))  # [128, 64]
    wf_np[:, 128:192] = Ap
    wf_np[:, 192:256] = W2f
    wf_np[:, 256:320] = W2g
    wf_np[:, 320:384] = W2x
    wf_np[:, 384:448] = W2y
    wf_np[0:8, 512:576] = Ah
    wf_np[0:8, 576:640] = W1x[0:8]
    wf_np[0:8, 640:704] = W1x[8:16]
    wf_np[0:8, 704:768] = W1x[16:24]
    wf_np[0:8, 768:832] = W1x[24:32]

    bias_np = np.zeros((128, 16), np.float32)
    bias_np[0:64, 0] = b1f + b2g @ D       # y is stored shifted by -b2g
    bias_np[0:64, 1] = b1g
    bias_np[0:64, 2] = b2f
    bias_np[0:64, 3] = b1x
    bias_np[0:64, 4] = b2x
    bias_np[0:64, 5] = b1y
    bias_np[64:128, 6] = b2y - b2g
    bias_np[64:128, 7] = b2g

    goal_np = goal.reshape(8, 1).astype(np.float32)
    in_maps = []
    for k in range(CORES):
        in_maps.append(dict(
            vct=vct_np,
            sgidx=_wrap_idx(slot_src[k].astype(np.int16)),
            vcpT=np.ascontiguousarray(vcp[perm[k]].T),
            goal=goal_np,
            wb=wb_np,
            wf=wf_np,
            wf2=wf2_np,
            bias=bias_np,
        ))
    return NI, LOOP, in_maps, slot_real, perm, src, tgt, N


def kernel(**inputs):
    global LAST_RESULT
    from concourse import bass_utils

    NI, LOOP, in_maps, slot_real, perm, src, tgt, N = _host_prep(inputs)
    key = (NI, LOOP)
    if key not in _CACHE:
        _CACHE[key] = _build_bass_program(0, LOOP)
    nc = _CACHE[key]

    res = bass_utils.run_bass_kernel_spmd(
        nc, in_maps, core_ids=list(range(CORES)), trace=PROFILE)
    LAST_RESULT = res

    edge_feat = np.zeros((N, N, E), np.float32)
    x_full = np.zeros((NPAD, 64), np.float32)
    for k in range(CORES):
        yk = np.ascontiguousarray(res.results[k]["y_out"].T)   # [NI, 64]
        real = slot_real[k] >= 0
        eids = slot_real[k][real]
        edge_feat[src[eids], tgt[eids]] = yk[real]
        x_full[perm[k]] = res.results[k]["x_out"].T            # undo relabeling
    return edge_feat, x_full[:N]


# revision 25
# speedup vs baseline: 1.0707x; 1.0707x over previous
"""Trainium2 Bass kernel for nn_GNNet_5420248728073 (GNN message passing).

Strategy (8 NeuronCores, SPMD):
  - Nodes are sharded into 8 blocks of 192 (N=1500 padded to 1536); core k owns
    the edges whose *target* lies in its block, laid out on a per-node slot grid
    [192 nodes x L slots] so segment-max becomes a fixed-stride tensor_reduce.
  - Per loop iteration each core computes fx messages for its edges, reduces to
    agg over its own nodes (all edges of a target live on one core), updates its
    x block, then AllGathers the x blocks (as bf16 main+residual rows) into a
    replicated node table, from which x[src] is fetched with a transposed
    dma_gather (feature-major).  x[tgt] is a broadcast expand of the local
    block (exact f32).  fy then updates the per-edge state y.
  - Precision: gathered x travels as bf16 (m) + bf16 residual (r); matmuls use
    split weights so the result is f32-accurate (~1e-5 rel).  Everything else
    (h, second layers, y, msg, agg, x) is f32.
  - Output: y rows (feature-major) + x block per core; the host unshard places
    the 24000 y rows into the (zero) dense [N, N, 64] edge_feat.
"""

import numpy as np
import ml_dtypes

E = 64            # feature dim
NB = 192          # nodes per core block
CORES = 8
NPAD = NB * CORES # 1536


def _bf16(a):
    return np.asarray(a, np.float32).astype(ml_dtypes.bfloat16)


def _wrap_idx(idx):
    """dma_gather index layout: [128, n/16] int16, idx i at [i%16, i//16],
    replicated across the 8 gpsimd cores (16 partitions each)."""
    n = idx.shape[0]
    assert n % 16 == 0
    base = idx.reshape(n // 16, 16).T.astype(np.int16)   # [16, n/16]
    return np.tile(base, (8, 1))                          # [128, n/16]


def _build_bass_program(L, LOOP, stage=99):
    LM, LX = 24, 16
    import concourse.bass as bass
    import concourse.bacc as bacc
    import concourse.tile as tile
    import concourse.mybir as mybir
    from concourse.masks import make_identity

    NI = NB * LM + 32 * LX
    NCH = NI // 512
    assert NI % 512 == 0 and NI % 128 == 0

    f32 = mybir.dt.float32
    bf = mybir.dt.bfloat16
    AF = mybir.ActivationFunctionType
    OP = mybir.AluOpType

    nc = bacc.Bacc("TRN2", target_bir_lowering=False, debug=False,
                   num_devices=CORES)

    # ---- I/O ----
    vct = nc.dram_tensor("vct", [NPAD, 128], bf, kind="ExternalInput")
    sgidx = nc.dram_tensor("sgidx", [128, NI // 16], mybir.dt.int16, kind="ExternalInput")
    vcpT = nc.dram_tensor("vcpT", [8, NB], f32, kind="ExternalInput")
    goal = nc.dram_tensor("goal", [8, 1], f32, kind="ExternalInput")
    wb = nc.dram_tensor("wb", [128, 384], bf, kind="ExternalInput")
    wf = nc.dram_tensor("wf", [64, 832], f32, kind="ExternalInput")
    wf2 = nc.dram_tensor("wf2", [128, 64], f32, kind="ExternalInput")
    bias = nc.dram_tensor("bias", [128, 16], f32, kind="ExternalInput")
    y_out = nc.dram_tensor("y_out", [64, NI], f32, kind="ExternalOutput")
    x_out = nc.dram_tensor("x_out", [64, NB], f32, kind="ExternalOutput")

    with tile.TileContext(nc) as tc:
        with (
            tc.tile_pool(name="persist", bufs=1) as pp,
            tc.tile_pool(name="rot", bufs=2) as rot,
            tc.tile_pool(name="work", bufs=3) as wk,
            tc.tile_pool(name="psum", bufs=2, space="PSUM") as ps,
            tc.tile_pool(name="dram", bufs=2, space="DRAM") as dr,
        ):
            # ---- load constants ----
            wb_sb = pp.tile([128, 384], bf, tag="wb")
            nc.sync.dma_start(wb_sb[:], wb[:])
            wf_sb = pp.tile([64, 832], f32, tag="wf")
            nc.sync.dma_start(wf_sb[:], wf[:])
            wf_sb2 = pp.tile([128, 64], f32, tag="wf2")
            nc.sync.dma_start(wf_sb2[:], wf2[:])
            b_sb = pp.tile([128, 16], f32, tag="bias")
            nc.sync.dma_start(b_sb[:], bias[:])
            idx_sb = pp.tile([128, NI // 16], mybir.dt.int16, tag="idx")
            nc.sync.dma_start(idx_sb[:], sgidx[:])
            vcT_sb = pp.tile([8, NB], f32, tag="vcT")
            nc.sync.dma_start(vcT_sb[:], vcpT[:])
            goal_sb = pp.tile([8, 1], f32, tag="goal")
            nc.sync.dma_start(goal_sb[:], goal[:])
            ident = pp.tile([128, 128], bf, tag="ident")
            make_identity(nc, ident[:])

            W_fx1 = wb_sb[:, 0:64]
            W_fx2 = wb_sb[:, 64:128]
            W_fy1 = wb_sb[:, 128:192]
            W_fy2 = wb_sb[:, 192:256]
            W_hy1 = wb_sb[0:16, 256:320]
            W_hy2 = wb_sb[0:8, 320:384]
            W_BD = wf_sb2[:, 0:64]
            W_Ap = wf_sb[:, 128:192]
            W_2f = wf_sb[:, 192:256]
            W_2g = wf_sb[:, 256:320]
            W_2x = wf_sb[:, 320:384]
            W_2y = wf_sb[:, 384:448]
            W_Ah = wf_sb[0:8, 512:576]
            W_1xa = wf_sb[0:8, 576:640]
            W_1xb = wf_sb[0:8, 640:704]
            W_1xc = wf_sb[0:8, 704:768]
            W_1xd = wf_sb[0:8, 768:832]
            b1f = b_sb[0:64, 0:1]
            b1g = b_sb[0:64, 1:2]
            b2f = b_sb[0:64, 2:3]
            b1x = b_sb[0:64, 3:4]
            b2x = b_sb[0:64, 4:5]
            b1y = b_sb[0:64, 5:6]
            b2ys = b_sb[64:128, 6:7]
            b2g_hi = b_sb[64:128, 7:8]

            # ---- persistent state ----
            x_blk = pp.tile([64, NB], f32, tag="x")        # f32 x of my block
            U = pp.tile([128, NI], f32, tag="U")           # [T=x[tgt]; y-b2g]
            grid = pp.tile([128, NI], f32, tag="grid")     # msg slots (lo half)
            agg = pp.tile([64, NB], f32, tag="agg")
            agg_e = pp.tile([64, 32], f32, tag="agge")
            NMAIN = NB * LM                                # main slots
            assert NMAIN + 32 * LX == NI

            # ---- x0 = mlp_hx([vc, goal, d, d*d]) for my block ----
            t_goal = pp.tile([8, NB], f32, tag="tgoal")
            nc.vector.tensor_copy(t_goal[:], goal_sb[:].to_broadcast([8, NB]))
            t_d = pp.tile([8, NB], f32, tag="td")
            nc.vector.tensor_scalar_sub(t_d[:], vcT_sb[:], goal_sb[:])
            t_dd = pp.tile([8, NB], f32, tag="tdd")
            nc.vector.tensor_tensor(out=t_dd[:], in0=t_d[:], in1=t_d[:], op=OP.mult)
            ph = ps.tile([64, NB], f32, tag="pa")
            nc.tensor.matmul(ph[:], W_1xa, vcT_sb[:], start=True, stop=False)
            nc.tensor.matmul(ph[:], W_1xb, t_goal[:], start=False, stop=False)
            nc.tensor.matmul(ph[:], W_1xc, t_d[:], start=False, stop=False)
            nc.tensor.matmul(ph[:], W_1xd, t_dd[:], start=False, stop=True)
            h0 = wk.tile([64, NB], f32, tag="h")
            nc.scalar.activation(h0[:], ph[:], AF.Relu, bias=b1x)
            px = ps.tile([64, NB], f32, tag="pb")
            nc.tensor.matmul(px[:], W_2x, h0[:], start=True, stop=True)
            nc.scalar.activation(x_blk[:], px[:], AF.Identity, bias=b2x)

            def x_chain():
                """cast x_blk -> [m|r] node-major rows, AllGather into the
                replicated table, gather S1=[m;r][src] and expand T=x[tgt]."""
                xm = wk.tile([64, NB], bf, tag="xm")
                nc.scalar.activation(xm[:], x_blk[:], AF.Copy)
                xr_f = wk.tile([64, NB], f32, tag="xrf")
                nc.vector.tensor_tensor(out=xr_f[:], in0=x_blk[:], in1=xm[:],
                                        op=OP.subtract)
                xr = wk.tile([64, NB], bf, tag="xr")
                nc.scalar.activation(xr[:], xr_f[:], AF.Copy)
                nmA = wk.tile([128, 128], bf, tag="nmA")
                nmB = wk.tile([64, 128], bf, tag="nmB")
                for srcap, dlo in ((xm, 0), (xr, 64)):
                    pt = ps.tile([128, 64], bf, tag="pa")
                    nc.tensor.transpose(pt[0:128, 0:64], srcap[:, 0:128],
                                        ident[0:64, 0:64])
                    nc.vector.tensor_copy(nmA[:, dlo:dlo + 64], pt[0:128, 0:64])
                    pt2 = ps.tile([64, 64], bf, tag="pb")
                    nc.tensor.transpose(pt2[0:64, 0:64], srcap[:, 128:192],
                                        ident[0:64, 0:64])
                    nc.vector.tensor_copy(nmB[:, dlo:dlo + 64], pt2[0:64, 0:64])
                agin = dr.tile([NB, 128], bf, tag="agin")
                nc.sync.dma_start(agin[0:128, :], nmA[:])
                nc.sync.dma_start(agin[128:192, :], nmB[:])
                table = dr.tile([NPAD, 128], bf, tag="table")
                nc.gpsimd.collective_compute(
                    "AllGather", OP.bypass,
                    replica_groups=[list(range(CORES))],
                    ins=[agin.opt()], outs=[table.opt()],
                )
                S1 = rot.tile([128, NI], bf, tag="S1")
                H = NI // 2
                for hh in range(2):
                    nc.gpsimd.dma_gather(
                        S1[:, hh * H:(hh + 1) * H].rearrange("p (a n) -> p a n", a=1),
                        table[:], idx_sb[:, hh * (H // 16):(hh + 1) * (H // 16)],
                        H, H, 128, transpose=True, single_packet=False,
                    )
                nc.vector.tensor_copy(
                    U[0:64, 0:NMAIN].rearrange("p (n l) -> p n l", l=LM),
                    x_blk[:, :, None].to_broadcast([64, NB, LM]),
                )
                nc.vector.tensor_copy(
                    U[0:64, NMAIN:NI].rearrange("p (n l) -> p n l", l=LX),
                    x_blk[:, 0:32, None].to_broadcast([64, 32, LX]),
                )
                return S1

            # ---- y0 (uses vc table: [vm|vr] in first 16 halfwords) ----
            G0 = rot.tile([128, NI], bf, tag="S1")
            H = NI // 2
            for hh in range(2):
                nc.gpsimd.dma_gather(
                    G0[:, hh * H:(hh + 1) * H].rearrange("p (a n) -> p a n", a=1),
                    vct[:], idx_sb[:, hh * (H // 16):(hh + 1) * (H // 16)],
                    H, H, 128, transpose=True, single_packet=False,
                )
            T0 = pp.tile([8, NI], f32, tag="T0")
            nc.vector.tensor_copy(
                T0[:, 0:NMAIN].rearrange("p (n l) -> p n l", l=LM),
                vcT_sb[:, :, None].to_broadcast([8, NB, LM]),
            )
            nc.vector.tensor_copy(
                T0[:, NMAIN:NI].rearrange("p (n l) -> p n l", l=LX),
                vcT_sb[:, 0:32, None].to_broadcast([8, 32, LX]),
            )
            for c in range(NCH):
                cs = slice(c * 512, (c + 1) * 512)
                p1 = ps.tile([64, 512], f32, tag="pa")
                nc.tensor.matmul(p1[:], W_hy1, G0[0:16, cs], start=True, stop=False)
                nc.tensor.matmul(p1[:], W_hy2, G0[0:8, cs], start=False, stop=False)
                nc.tensor.matmul(p1[:], W_Ah, T0[:, cs], start=False, stop=True)
                hh = wk.tile([64, 512], f32, tag="h")
                nc.scalar.activation(hh[:], p1[:], AF.Relu, bias=b1y)
                p2 = ps.tile([128, 512], f32, tag="pb")
                nc.tensor.matmul(p2[64:128, :], W_2y, hh[:], start=True, stop=True,
                                 tile_position=(0, 64))
                nc.scalar.activation(U[64:128, cs], p2[64:128, :], AF.Identity,
                                     bias=b2ys)

            S1 = x_chain()

            # ---- main loop ----
            for t in range(LOOP if stage >= 6 else 0):
                # fx: msg into grid (lo half)
                for c in range(NCH):
                    cs = slice(c * 512, (c + 1) * 512)
                    p1 = ps.tile([64, 512], f32, tag="pa")
                    nc.tensor.matmul(p1[:], W_fx1, S1[:, cs], start=True, stop=False)
                    nc.tensor.matmul(p1[:], W_fx2, S1[:, cs], start=False, stop=False)
                    nc.tensor.matmul(p1[:], W_BD, U[:, cs], start=False, stop=True)
                    hh = wk.tile([64, 512], f32, tag="h")
                    nc.scalar.activation(hh[:], p1[:], AF.Relu, bias=b1f)
                    p2 = ps.tile([64, 512], f32, tag="pb")
                    nc.tensor.matmul(p2[:], W_2f, hh[:], start=True, stop=True)
                    nc.scalar.activation(grid[0:64, cs], p2[:], AF.Identity, bias=b2f)
                # segment max + x update (main grid + overflow tier)
                nc.vector.tensor_reduce(
                    agg[:], grid[0:64, 0:NMAIN].rearrange("p (n l) -> p n l", l=LM),
                    axis=mybir.AxisListType.X, op=OP.max,
                )
                nc.vector.tensor_reduce(
                    agg_e[:], grid[0:64, NMAIN:NI].rearrange("p (n l) -> p n l", l=LX),
                    axis=mybir.AxisListType.X, op=OP.max,
                )
                nc.vector.tensor_tensor(out=agg[:, 0:32], in0=agg[:, 0:32],
                                        in1=agg_e[:], op=OP.max)
                nc.vector.tensor_tensor(out=x_blk[:], in0=x_blk[:], in1=agg[:],
                                        op=OP.max)
                # distribute x_{t+1}; gathers serve fy_t and fx_{t+1}
                S1 = x_chain()
                # fy: y = max(y, mlp)
                for c in range(NCH):
                    cs = slice(c * 512, (c + 1) * 512)
                    p3 = ps.tile([64, 512], f32, tag="pc")
                    nc.tensor.matmul(p3[:], W_Ap, U[0:64, cs], start=True, stop=False)
                    nc.tensor.matmul(p3[:], W_fy1, S1[:, cs], start=False, stop=False)
                    nc.tensor.matmul(p3[:], W_fy2, S1[:, cs], start=False, stop=True)
                    hh = wk.tile([64, 512], f32, tag="h")
                    nc.scalar.activation(hh[:], p3[:], AF.Relu, bias=b1g)
                    p4 = ps.tile([128, 512], f32, tag="pd")
                    nc.tensor.matmul(p4[64:128, :], W_2g, hh[:], start=True, stop=True,
                                     tile_position=(0, 64))
                    nc.vector.tensor_tensor(out=U[64:128, cs], in0=U[64:128, cs],
                                            in1=p4[64:128, :], op=OP.max)

            # ---- outputs ----
            nc.scalar.activation(grid[64:128, :], U[64:128, :], AF.Identity,
                                 bias=b2g_hi)
            nc.sync.dma_start(y_out[:], grid[64:128, :])
            nc.sync.dma_start(x_out[:], x_blk[:])

    nc.compile()
    return nc


_CACHE = {}
PROFILE = False       # set True (before calling kernel) to capture an NTFF trace
LAST_RESULT = None    # BassKernelResults of the last run


def _host_prep(inputs):
    inp = {k: np.asarray(v) for k, v in inputs.items()}
    v = inp["v"].astype(np.float32)
    labels = inp["labels"].astype(np.float32)
    edge_index = inp["edge_index"].astype(np.int64)
    LOOP = int(inp["loop"])
    N = v.shape[0]
    src, tgt = edge_index[0], edge_index[1]

    vc = np.concatenate([v, labels], -1)                    # [N, 8]
    goal = vc[int(np.argmax(labels[:, 0]))]
    vcp = np.zeros((NPAD, 8), np.float32)
    vcp[:N] = vc

    # per-target-node edge lists; two-tier slot grid:
    #   main: LM slots per node; ext: LX extra slots for local nodes 0..31
    #   (nodes are relabeled per block by degree descending)
    LM, LX = 24, 16
    NI = NB * LM + 32 * LX
    NMAIN = NB * LM
    deg = np.bincount(tgt, minlength=NPAD)
    assert int(deg.max()) <= LM + LX

    order = np.argsort(tgt, kind="stable")
    starts = np.zeros(NPAD + 1, np.int64)
    np.cumsum(deg, out=starts[1:])

    slot_src = np.zeros((CORES, NI), np.int64)
    slot_real = np.full((CORES, NI), -1, np.int64)   # original edge id or -1
    perm = np.zeros((CORES, NB), np.int64)           # local id -> global node
    local_of = np.zeros(NPAD, np.int64)              # global -> table row
    for k in range(CORES):
        blk_deg = deg[k * NB:(k + 1) * NB]
        assert (blk_deg > LM).sum() <= 32
        rank = np.argsort(-blk_deg, kind="stable")   # local id ln = rank order
        perm[k] = k * NB + rank
        local_of[perm[k]] = k * NB + np.arange(NB)
        for ln in range(NB):
            g = int(perm[k, ln])
            es = order[starts[g]:starts[g + 1]]
            dn = len(es)
            s0 = ln * LM
            if dn == 0:
                slot_src[k, s0:s0 + LM] = g
                continue
            n_main = min(dn, LM)
            slot_src[k, s0:s0 + n_main] = src[es[:n_main]]
            slot_real[k, s0:s0 + n_main] = es[:n_main]
            slot_src[k, s0 + n_main:s0 + LM] = src[es[0]]
            if ln < 32:
                e0 = NMAIN + ln * LX
                n_ext = dn - n_main
                slot_src[k, e0:e0 + LX] = src[es[0]]
                if n_ext > 0:
                    slot_src[k, e0:e0 + n_ext] = src[es[n_main:]]
                    slot_real[k, e0:e0 + n_ext] = es[n_main:]
            else:
                assert dn <= LM
    # gather indices address the (relabeled) table rows
    slot_src = local_of[slot_src]

    # vc main+residual gather table, rows in table (relabeled) order
    flat_perm = perm.reshape(-1)
    vcp_l = vcp[flat_perm]
    vm = _bf16(vcp_l)
    vr = _bf16(vcp_l - vm.astype(np.float32))
    vct_np = np.zeros((NPAD, 128), ml_dtypes.bfloat16)
    vct_np[:, 0:8] = vm
    vct_np[:, 8:16] = vr

    # weights
    W1x, b1x, W2x, b2x = inp["hx_W1"], inp["hx_b1"], inp["hx_W2"], inp["hx_b2"]
    W1y, b1y, W2y, b2y = inp["hy_W1"], inp["hy_b1"], inp["hy_W2"], inp["hy_b2"]
    W1f, b1f, W2f, b2f = inp["fx_W1"], inp["fx_b1"], inp["fx_W2"], inp["fx_b2"]
    W1g, b1g, W2g, b2g = inp["fy_W1"], inp["fy_b1"], inp["fy_W2"], inp["fy_b2"]
    A = W1f[64:128] + W1f[0:64]
    B = W1f[128:192] - W1f[0:64]
    D = W1f[192:256]
    Ap = W1g[0:64] + W1g[64:128]
    Bp = W1g[128:192] - W1g[0:64]
    Ah = W1y[0:8] + W1y[8:16]
    Bh = W1y[16:24] - W1y[0:8]

    Am = _bf16(A); Ar = _bf16(A - Am.astype(np.float32))
    Bpm = _bf16(Bp); Bpr = _bf16(Bp - Bpm.astype(np.float32))
    Bhm = _bf16(Bh); Bhr = _bf16(Bh - Bhm.astype(np.float32))

    wb_np = np.zeros((128, 384), ml_dtypes.bfloat16)
    wb_np[:, 0:64] = np.concatenate([Am, Am], 0)
    wb_np[:, 64:128] = np.concatenate([Ar, Ar], 0)
    wb_np[:, 128:192] = np.concatenate([Bpm, Bpm], 0)
    wb_np[:, 192:256] = np.concatenate([Bpr, Bpr], 0)
    wb_np[0:16, 256:320] = np.concatenate([Bhm, Bhm], 0)
    wb_np[0:8, 320:384] = Bhr

    wf_np = np.zeros((64, 832), np.float32)
    wf2_np = np.ascontiguousarray(np.concatenate([B, D], 0))  # [128, 64]
    wf_np[:, 128:192] = Ap
    wf_np[:, 192:256] = W2f
    wf_np[:, 256:320] = W2g
    wf_np[:, 320:384] = W2x
    wf_np[:, 384:448] = W2y
    wf_np[0:8, 512:576] = Ah
    wf_np[0:8, 576:640] = W1x[0:8]
    wf_np[0:8, 640:704] = W1x[8:16]
    wf_np[0:8, 704:768] = W1x[16:24]
    wf_np[0:8, 768:832] = W1x[24:32]

    bias_np = np.zeros((128, 16), np.float32)
    bias_np[0:64, 0] = b1f + b2g @ D       # y is stored shifted by -b2g
    bias_np[0:64, 1] = b1g
    bias_np[0:64, 2] = b2f
    bias_np[0:64, 3] = b1x
    bias_np[0:64, 4] = b2x
    bias_np[0:64, 5] = b1y
    bias_np[64:128, 6] = b2y - b2g
    bias_np[64:128, 7] = b2g

    goal_np = goal.reshape(8, 1).astype(np.float32)
    in_maps = []
    for k in range(CORES):
        in_maps.append(dict(
            vct=vct_np,
            sgidx=_wrap_idx(slot_src[k].astype(np.int16)),
            vcpT=np.ascontiguousarray(vcp[perm[k]].T),
            goal=goal_np,
            wb=wb_np,
            wf=wf_np,
            wf2=wf2_np,
            bias=bias_np,
        ))
    return NI, LOOP, in_maps, slot_real, perm, src, tgt, N


def kernel(**inputs):
    global LAST_RESULT
    from concourse import bass_utils

    NI, LOOP, in_maps, slot_real, perm, src, tgt, N = _host_prep(inputs)
    key = (NI, LOOP)
    if key not in _CACHE:
        _CACHE[key] = _build_bass_program(0, LOOP)
    nc = _CACHE[key]

    res = bass_utils.run_bass_kernel_spmd(
        nc, in_maps, core_ids=list(range(CORES)), trace=PROFILE)
    LAST_RESULT = res

    edge_feat = np.zeros((N, N, E), np.float32)
    x_full = np.zeros((NPAD, 64), np.float32)
    for k in range(CORES):
        yk = np.ascontiguousarray(res.results[k]["y_out"].T)   # [NI, 64]
        real = slot_real[k] >= 0
        eids = slot_real[k][real]
        edge_feat[src[eids], tgt[eids]] = yk[real]
        x_full[perm[k]] = res.results[k]["x_out"].T            # undo relabeling
    return edge_feat, x_full[:N]


# revision 26
# speedup vs baseline: 1.0856x; 1.0139x over previous
"""Trainium2 Bass kernel for nn_GNNet_5420248728073 (GNN message passing).

Strategy (8 NeuronCores, SPMD):
  - Nodes are sharded into 8 blocks of 192 (N=1500 padded to 1536); core k owns
    the edges whose *target* lies in its block, laid out on a per-node slot grid
    [192 nodes x L slots] so segment-max becomes a fixed-stride tensor_reduce.
  - Per loop iteration each core computes fx messages for its edges, reduces to
    agg over its own nodes (all edges of a target live on one core), updates its
    x block, then AllGathers the x blocks (as bf16 main+residual rows) into a
    replicated node table, from which x[src] is fetched with a transposed
    dma_gather (feature-major).  x[tgt] is a broadcast expand of the local
    block (exact f32).  fy then updates the per-edge state y.
  - Precision: gathered x travels as bf16 (m) + bf16 residual (r); matmuls use
    split weights so the result is f32-accurate (~1e-5 rel).  Everything else
    (h, second layers, y, msg, agg, x) is f32.
  - Output: y rows (feature-major) + x block per core; the host unshard places
    the 24000 y rows into the (zero) dense [N, N, 64] edge_feat.
"""

import numpy as np
import ml_dtypes

E = 64            # feature dim
NB = 192          # nodes per core block
CORES = 8
NPAD = NB * CORES # 1536


def _bf16(a):
    return np.asarray(a, np.float32).astype(ml_dtypes.bfloat16)


def _wrap_idx(idx):
    """dma_gather index layout: [128, n/16] int16, idx i at [i%16, i//16],
    replicated across the 8 gpsimd cores (16 partitions each)."""
    n = idx.shape[0]
    assert n % 16 == 0
    base = idx.reshape(n // 16, 16).T.astype(np.int16)   # [16, n/16]
    return np.tile(base, (8, 1))                          # [128, n/16]


def _build_bass_program(L, LOOP, stage=99):
    LM, LX = 24, 16
    import concourse.bass as bass
    import concourse.bacc as bacc
    import concourse.tile as tile
    import concourse.mybir as mybir
    from concourse.masks import make_identity

    NI = NB * LM + 32 * LX
    NCH = NI // 512
    assert NI % 512 == 0 and NI % 128 == 0

    f32 = mybir.dt.float32
    bf = mybir.dt.bfloat16
    AF = mybir.ActivationFunctionType
    OP = mybir.AluOpType

    nc = bacc.Bacc("TRN2", target_bir_lowering=False, debug=False,
                   num_devices=CORES)

    # ---- I/O ----
    vct = nc.dram_tensor("vct", [NPAD, 128], bf, kind="ExternalInput")
    sgidx = nc.dram_tensor("sgidx", [128, NI // 16], mybir.dt.int16, kind="ExternalInput")
    vcpT = nc.dram_tensor("vcpT", [8, NB], f32, kind="ExternalInput")
    goal = nc.dram_tensor("goal", [8, 1], f32, kind="ExternalInput")
    wb = nc.dram_tensor("wb", [128, 384], bf, kind="ExternalInput")
    wf = nc.dram_tensor("wf", [64, 832], f32, kind="ExternalInput")
    wf2 = nc.dram_tensor("wf2", [128, 64], f32, kind="ExternalInput")
    bias = nc.dram_tensor("bias", [128, 16], f32, kind="ExternalInput")
    y_out = nc.dram_tensor("y_out", [64, NI], f32, kind="ExternalOutput")
    x_out = nc.dram_tensor("x_out", [64, NB], f32, kind="ExternalOutput")

    with tile.TileContext(nc) as tc:
        with (
            tc.tile_pool(name="persist", bufs=1) as pp,
            tc.tile_pool(name="rot", bufs=2) as rot,
            tc.tile_pool(name="work", bufs=3) as wk,
            tc.tile_pool(name="psum", bufs=2, space="PSUM") as ps,
            tc.tile_pool(name="dram", bufs=2, space="DRAM") as dr,
        ):
            # ---- load constants ----
            wb_sb = pp.tile([128, 384], bf, tag="wb")
            nc.sync.dma_start(wb_sb[:], wb[:])
            wf_sb = pp.tile([64, 832], f32, tag="wf")
            nc.sync.dma_start(wf_sb[:], wf[:])
            wf_sb2 = pp.tile([128, 64], f32, tag="wf2")
            nc.sync.dma_start(wf_sb2[:], wf2[:])
            b_sb = pp.tile([128, 16], f32, tag="bias")
            nc.sync.dma_start(b_sb[:], bias[:])
            idx_sb = pp.tile([128, NI // 16], mybir.dt.int16, tag="idx")
            nc.sync.dma_start(idx_sb[:], sgidx[:])
            vcT_sb = pp.tile([8, NB], f32, tag="vcT")
            nc.sync.dma_start(vcT_sb[:], vcpT[:])
            goal_sb = pp.tile([8, 1], f32, tag="goal")
            nc.sync.dma_start(goal_sb[:], goal[:])
            ident = pp.tile([128, 128], bf, tag="ident")
            make_identity(nc, ident[:])

            W_fx1 = wb_sb[:, 0:64]
            W_fx2 = wb_sb[:, 64:128]
            W_fy1 = wb_sb[:, 128:192]
            W_fy2 = wb_sb[:, 192:256]
            W_hy1 = wb_sb[0:16, 256:320]
            W_hy2 = wb_sb[0:8, 320:384]
            W_BD = wf_sb2[:, 0:64]
            W_Ap = wf_sb[:, 128:192]
            W_2f = wf_sb[:, 192:256]
            W_2g = wf_sb[:, 256:320]
            W_2x = wf_sb[:, 320:384]
            W_2y = wf_sb[:, 384:448]
            W_Ah = wf_sb[0:8, 512:576]
            W_1xa = wf_sb[0:8, 576:640]
            W_1xb = wf_sb[0:8, 640:704]
            W_1xc = wf_sb[0:8, 704:768]
            W_1xd = wf_sb[0:8, 768:832]
            b1f = b_sb[0:64, 0:1]
            b1g = b_sb[0:64, 1:2]
            b2f = b_sb[0:64, 2:3]
            b1x = b_sb[0:64, 3:4]
            b2x = b_sb[0:64, 4:5]
            b1y = b_sb[0:64, 5:6]
            b2ys = b_sb[64:128, 6:7]
            b2g_hi = b_sb[64:128, 7:8]

            # ---- persistent state ----
            x_blk = pp.tile([64, NB], f32, tag="x")        # f32 x of my block
            U = pp.tile([128, NI], f32, tag="U")           # [T=x[tgt]; y-b2g]
            grid = pp.tile([128, NI], f32, tag="grid")     # msg slots (lo half)
            agg = pp.tile([64, NB], f32, tag="agg")
            agg_e = pp.tile([64, 32], f32, tag="agge")
            NMAIN = NB * LM                                # main slots
            assert NMAIN + 32 * LX == NI

            # ---- x0 = mlp_hx([vc, goal, d, d*d]) for my block ----
            t_goal = pp.tile([8, NB], f32, tag="tgoal")
            nc.vector.tensor_copy(t_goal[:], goal_sb[:].to_broadcast([8, NB]))
            t_d = pp.tile([8, NB], f32, tag="td")
            nc.vector.tensor_scalar_sub(t_d[:], vcT_sb[:], goal_sb[:])
            t_dd = pp.tile([8, NB], f32, tag="tdd")
            nc.vector.tensor_tensor(out=t_dd[:], in0=t_d[:], in1=t_d[:], op=OP.mult)
            ph = ps.tile([64, NB], f32, tag="pa")
            nc.tensor.matmul(ph[:], W_1xa, vcT_sb[:], start=True, stop=False)
            nc.tensor.matmul(ph[:], W_1xb, t_goal[:], start=False, stop=False)
            nc.tensor.matmul(ph[:], W_1xc, t_d[:], start=False, stop=False)
            nc.tensor.matmul(ph[:], W_1xd, t_dd[:], start=False, stop=True)
            h0 = wk.tile([64, NB], f32, tag="h")
            nc.scalar.activation(h0[:], ph[:], AF.Relu, bias=b1x)
            px = ps.tile([64, NB], f32, tag="pb")
            nc.tensor.matmul(px[:], W_2x, h0[:], start=True, stop=True)
            nc.scalar.activation(x_blk[:], px[:], AF.Identity, bias=b2x)

            def x_chain():
                """cast x_blk -> [m|r] node-major rows, AllGather into the
                replicated table, gather S1=[m;r][src] and expand T=x[tgt]."""
                xm = wk.tile([64, NB], bf, tag="xm")
                nc.scalar.activation(xm[:], x_blk[:], AF.Copy)
                xr_f = wk.tile([64, NB], f32, tag="xrf")
                nc.vector.tensor_tensor(out=xr_f[:], in0=x_blk[:], in1=xm[:],
                                        op=OP.subtract)
                xr = wk.tile([64, NB], bf, tag="xr")
                nc.scalar.activation(xr[:], xr_f[:], AF.Copy)
                nmA = wk.tile([128, 128], bf, tag="nmA")
                nmB = wk.tile([64, 128], bf, tag="nmB")
                for srcap, dlo in ((xm, 0), (xr, 64)):
                    pt = ps.tile([128, 64], bf, tag="pa")
                    nc.tensor.transpose(pt[0:128, 0:64], srcap[:, 0:128],
                                        ident[0:64, 0:64])
                    nc.vector.tensor_copy(nmA[:, dlo:dlo + 64], pt[0:128, 0:64])
                    pt2 = ps.tile([64, 64], bf, tag="pb")
                    nc.tensor.transpose(pt2[0:64, 0:64], srcap[:, 128:192],
                                        ident[0:64, 0:64])
                    nc.vector.tensor_copy(nmB[:, dlo:dlo + 64], pt2[0:64, 0:64])
                agin = dr.tile([NB, 128], bf, tag="agin")
                nc.sync.dma_start(agin[0:128, :], nmA[:])
                nc.sync.dma_start(agin[128:192, :], nmB[:])
                table = dr.tile([NPAD, 128], bf, tag="table")
                nc.gpsimd.collective_compute(
                    "AllGather", OP.bypass,
                    replica_groups=[list(range(CORES))],
                    ins=[agin.opt()], outs=[table.opt()],
                )
                S1 = rot.tile([128, NI], bf, tag="S1")
                H = NI // 2
                for hh in range(2):
                    nc.gpsimd.dma_gather(
                        S1[:, hh * H:(hh + 1) * H].rearrange("p (a n) -> p a n", a=1),
                        table[:], idx_sb[:, hh * (H // 16):(hh + 1) * (H // 16)],
                        H, H, 128, transpose=True, single_packet=False,
                    )
                nc.vector.tensor_copy(
                    U[0:64, 0:NMAIN].rearrange("p (n l) -> p n l", l=LM),
                    x_blk[:, :, None].to_broadcast([64, NB, LM]),
                )
                nc.vector.tensor_copy(
                    U[0:64, NMAIN:NI].rearrange("p (n l) -> p n l", l=LX),
                    x_blk[:, 0:32, None].to_broadcast([64, 32, LX]),
                )
                return S1

            # ---- y0 (uses vc table: [vm|vr] in first 16 halfwords) ----
            G0 = rot.tile([128, NI], bf, tag="S1")
            H = NI // 2
            for hh in range(2):
                nc.gpsimd.dma_gather(
                    G0[:, hh * H:(hh + 1) * H].rearrange("p (a n) -> p a n", a=1),
                    vct[:], idx_sb[:, hh * (H // 16):(hh + 1) * (H // 16)],
                    H, H, 128, transpose=True, single_packet=False,
                )
            T0 = pp.tile([8, NI], f32, tag="T0")
            nc.vector.tensor_copy(
                T0[:, 0:NMAIN].rearrange("p (n l) -> p n l", l=LM),
                vcT_sb[:, :, None].to_broadcast([8, NB, LM]),
            )
            nc.vector.tensor_copy(
                T0[:, NMAIN:NI].rearrange("p (n l) -> p n l", l=LX),
                vcT_sb[:, 0:32, None].to_broadcast([8, 32, LX]),
            )
            for c in range(NCH):
                cs = slice(c * 512, (c + 1) * 512)
                p1 = ps.tile([64, 512], f32, tag="pa")
                nc.tensor.matmul(p1[:], W_hy1, G0[0:16, cs], start=True, stop=False)
                nc.tensor.matmul(p1[:], W_hy2, G0[0:8, cs], start=False, stop=False)
                nc.tensor.matmul(p1[:], W_Ah, T0[:, cs], start=False, stop=True)
                hh = wk.tile([64, 512], f32, tag="h")
                nc.scalar.activation(hh[:], p1[:], AF.Relu, bias=b1y)
                p2 = ps.tile([128, 512], f32, tag="pb")
                nc.tensor.matmul(p2[64:128, :], W_2y, hh[:], start=True, stop=True,
                                 tile_position=(0, 64))
                nc.scalar.activation(U[64:128, cs], p2[64:128, :], AF.Identity,
                                     bias=b2ys)

            S1 = x_chain()

            # ---- main loop ----
            for t in range(LOOP if stage >= 6 else 0):
                # fx: msg into grid (lo half)
                for c in range(NCH):
                    cs = slice(c * 512, (c + 1) * 512)
                    p1 = ps.tile([64, 512], f32, tag="pa")
                    nc.tensor.matmul(p1[:], W_fx1, S1[:, cs], start=True, stop=False)
                    nc.tensor.matmul(p1[:], W_fx2, S1[:, cs], start=False, stop=False)
                    nc.tensor.matmul(p1[:], W_BD, U[:, cs], start=False, stop=True)
                    hh = wk.tile([64, 512], f32, tag="h")
                    nc.scalar.activation(hh[:], p1[:], AF.Relu, bias=b1f)
                    p2 = ps.tile([64, 512], f32, tag="pb")
                    nc.tensor.matmul(p2[:], W_2f, hh[:], start=True, stop=True)
                    nc.scalar.activation(grid[0:64, cs], p2[:], AF.Identity, bias=b2f)
                # segment max + x update (main grid + overflow tier)
                nc.vector.tensor_reduce(
                    agg[:], grid[0:64, 0:NMAIN].rearrange("p (n l) -> p n l", l=LM),
                    axis=mybir.AxisListType.X, op=OP.max,
                )
                nc.vector.tensor_reduce(
                    agg_e[:], grid[0:64, NMAIN:NI].rearrange("p (n l) -> p n l", l=LX),
                    axis=mybir.AxisListType.X, op=OP.max,
                )
                nc.vector.tensor_tensor(out=agg[:, 0:32], in0=agg[:, 0:32],
                                        in1=agg_e[:], op=OP.max)
                nc.vector.tensor_tensor(out=x_blk[:], in0=x_blk[:], in1=agg[:],
                                        op=OP.max)
                # distribute x_{t+1}; gathers serve fy_t and fx_{t+1}
                S1 = x_chain()
                # fy: y = max(y, mlp)
                for c in range(NCH):
                    cs = slice(c * 512, (c + 1) * 512)
                    p3 = ps.tile([64, 512], f32, tag=("pc" if c % 2 else "pa"))
                    nc.tensor.matmul(p3[:], W_Ap, U[0:64, cs], start=True, stop=False)
                    nc.tensor.matmul(p3[:], W_fy1, S1[:, cs], start=False, stop=False)
                    nc.tensor.matmul(p3[:], W_fy2, S1[:, cs], start=False, stop=True)
                    hh = wk.tile([64, 512], f32, tag="h")
                    nc.scalar.activation(hh[:], p3[:], AF.Relu, bias=b1g)
                    p4 = ps.tile([128, 512], f32, tag=("pd" if c % 2 else "pb"))
                    nc.tensor.matmul(p4[64:128, :], W_2g, hh[:], start=True, stop=True,
                                     tile_position=(0, 64))
                    nc.vector.tensor_tensor(out=U[64:128, cs], in0=U[64:128, cs],
                                            in1=p4[64:128, :], op=OP.max)

            # ---- outputs ----
            nc.scalar.activation(grid[64:128, :], U[64:128, :], AF.Identity,
                                 bias=b2g_hi)
            nc.sync.dma_start(y_out[:], grid[64:128, :])
            nc.sync.dma_start(x_out[:], x_blk[:])

    nc.compile()
    return nc


_CACHE = {}
PROFILE = False       # set True (before calling kernel) to capture an NTFF trace
LAST_RESULT = None    # BassKernelResults of the last run


def _host_prep(inputs):
    inp = {k: np.asarray(v) for k, v in inputs.items()}
    v = inp["v"].astype(np.float32)
    labels = inp["labels"].astype(np.float32)
    edge_index = inp["edge_index"].astype(np.int64)
    LOOP = int(inp["loop"])
    N = v.shape[0]
    src, tgt = edge_index[0], edge_index[1]

    vc = np.concatenate([v, labels], -1)                    # [N, 8]
    goal = vc[int(np.argmax(labels[:, 0]))]
    vcp = np.zeros((NPAD, 8), np.float32)
    vcp[:N] = vc

    # per-target-node edge lists; two-tier slot grid:
    #   main: LM slots per node; ext: LX extra slots for local nodes 0..31
    #   (nodes are relabeled per block by degree descending)
    LM, LX = 24, 16
    NI = NB * LM + 32 * LX
    NMAIN = NB * LM
    deg = np.bincount(tgt, minlength=NPAD)
    assert int(deg.max()) <= LM + LX

    order = np.argsort(tgt, kind="stable")
    starts = np.zeros(NPAD + 1, np.int64)
    np.cumsum(deg, out=starts[1:])

    slot_src = np.zeros((CORES, NI), np.int64)
    slot_real = np.full((CORES, NI), -1, np.int64)   # original edge id or -1
    perm = np.zeros((CORES, NB), np.int64)           # local id -> global node
    local_of = np.zeros(NPAD, np.int64)              # global -> table row
    for k in range(CORES):
        blk_deg = deg[k * NB:(k + 1) * NB]
        assert (blk_deg > LM).sum() <= 32
        rank = np.argsort(-blk_deg, kind="stable")   # local id ln = rank order
        perm[k] = k * NB + rank
        local_of[perm[k]] = k * NB + np.arange(NB)
        for ln in range(NB):
            g = int(perm[k, ln])
            es = order[starts[g]:starts[g + 1]]
            dn = len(es)
            s0 = ln * LM
            if dn == 0:
                slot_src[k, s0:s0 + LM] = g
                continue
            n_main = min(dn, LM)
            slot_src[k, s0:s0 + n_main] = src[es[:n_main]]
            slot_real[k, s0:s0 + n_main] = es[:n_main]
            slot_src[k, s0 + n_main:s0 + LM] = src[es[0]]
            if ln < 32:
                e0 = NMAIN + ln * LX
                n_ext = dn - n_main
                slot_src[k, e0:e0 + LX] = src[es[0]]
                if n_ext > 0:
                    slot_src[k, e0:e0 + n_ext] = src[es[n_main:]]
                    slot_real[k, e0:e0 + n_ext] = es[n_main:]
            else:
                assert dn <= LM
    # gather indices address the (relabeled) table rows
    slot_src = local_of[slot_src]

    # vc main+residual gather table, rows in table (relabeled) order
    flat_perm = perm.reshape(-1)
    vcp_l = vcp[flat_perm]
    vm = _bf16(vcp_l)
    vr = _bf16(vcp_l - vm.astype(np.float32))
    vct_np = np.zeros((NPAD, 128), ml_dtypes.bfloat16)
    vct_np[:, 0:8] = vm
    vct_np[:, 8:16] = vr

    # weights
    W1x, b1x, W2x, b2x = inp["hx_W1"], inp["hx_b1"], inp["hx_W2"], inp["hx_b2"]
    W1y, b1y, W2y, b2y = inp["hy_W1"], inp["hy_b1"], inp["hy_W2"], inp["hy_b2"]
    W1f, b1f, W2f, b2f = inp["fx_W1"], inp["fx_b1"], inp["fx_W2"], inp["fx_b2"]
    W1g, b1g, W2g, b2g = inp["fy_W1"], inp["fy_b1"], inp["fy_W2"], inp["fy_b2"]
    A = W1f[64:128] + W1f[0:64]
    B = W1f[128:192] - W1f[0:64]
    D = W1f[192:256]
    Ap = W1g[0:64] + W1g[64:128]
    Bp = W1g[128:192] - W1g[0:64]
    Ah = W1y[0:8] + W1y[8:16]
    Bh = W1y[16:24] - W1y[0:8]

    Am = _bf16(A); Ar = _bf16(A - Am.astype(np.float32))
    Bpm = _bf16(Bp); Bpr = _bf16(Bp - Bpm.astype(np.float32))
    Bhm = _bf16(Bh); Bhr = _bf16(Bh - Bhm.astype(np.float32))

    wb_np = np.zeros((128, 384), ml_dtypes.bfloat16)
    wb_np[:, 0:64] = np.concatenate([Am, Am], 0)
    wb_np[:, 64:128] = np.concatenate([Ar, Ar], 0)
    wb_np[:, 128:192] = np.concatenate([Bpm, Bpm], 0)
    wb_np[:, 192:256] = np.concatenate([Bpr, Bpr], 0)
    wb_np[0:16, 256:320] = np.concatenate([Bhm, Bhm], 0)
    wb_np[0:8, 320:384] = Bhr

    wf_np = np.zeros((64, 832), np.float32)
    wf2_np = np.ascontiguousarray(np.concatenate([B, D], 0))  # [128, 64]
    wf_np[:, 128:192] = Ap
    wf_np[:, 192:256] = W2f
    wf_np[:, 256:320] = W2g
    wf_np[:, 320:384] = W2x
    wf_np[:, 384:448] = W2y
    wf_np[0:8, 512:576] = Ah
    wf_np[0:8, 576:640] = W1x[0:8]
    wf_np[0:8, 640:704] = W1x[8:16]
    wf_np[0:8, 704:768] = W1x[16:24]
    wf_np[0:8, 768:832] = W1x[24:32]

    bias_np = np.zeros((128, 16), np.float32)
    bias_np[0:64, 0] = b1f + b2g @ D       # y is stored shifted by -b2g
    bias_np[0:64, 1] = b1g
    bias_np[0:64, 2] = b2f
    bias_np[0:64, 3] = b1x
    bias_np[0:64, 4] = b2x
    bias_np[0:64, 5] = b1y
    bias_np[64:128, 6] = b2y - b2g
    bias_np[64:128, 7] = b2g

    goal_np = goal.reshape(8, 1).astype(np.float32)
    in_maps = []
    for k in range(CORES):
        in_maps.append(dict(
            vct=vct_np,
            sgidx=_wrap_idx(slot_src[k].astype(np.int16)),
            vcpT=np.ascontiguousarray(vcp[perm[k]].T),
            goal=goal_np,
            wb=wb_np,
            wf=wf_np,
            wf2=wf2_np,
            bias=bias_np,
        ))
    return NI, LOOP, in_maps, slot_real, perm, src, tgt, N


def kernel(**inputs):
    global LAST_RESULT
    from concourse import bass_utils

    NI, LOOP, in_maps, slot_real, perm, src, tgt, N = _host_prep(inputs)
    key = (NI, LOOP)
    if key not in _CACHE:
        _CACHE[key] = _build_bass_program(0, LOOP)
    nc = _CACHE[key]

    res = bass_utils.run_bass_kernel_spmd(
        nc, in_maps, core_ids=list(range(CORES)), trace=PROFILE)
    LAST_RESULT = res

    edge_feat = np.zeros((N, N, E), np.float32)
    x_full = np.zeros((NPAD, 64), np.float32)
    for k in range(CORES):
        yk = np.ascontiguousarray(res.results[k]["y_out"].T)   # [NI, 64]
        real = slot_real[k] >= 0
        eids = slot_real[k][real]
        edge_feat[src[eids], tgt[eids]] = yk[real]
        x_full[perm[k]] = res.results[k]["x_out"].T            # undo relabeling
    return edge_feat, x_full[:N]
